# revision 12
# baseline (speedup 1.0000x reference)
"""Trainium2 Bass kernel for nn_ConsistLoss (retrieval_knn).

Math notes
----------
reference() = mean(|rigid_refine - pred^T|) where
  rigid_refine = rigid_recon - mean_i(laplace_x_i - laplace_y_i)
  laplace_c_i  = (sum_{j in 6NN_c(i)} c_j - 6*q_i) / 5       (c in {x=rigid_recon, y})
The -6*q_i terms cancel in (laplace_x - laplace_y), and only the MEAN over all
i is needed, so:
  mean_vec = ( sum_j cx(j)*x_j - sum_j cy(j)*y_j ) / (5*N)
where cx(j) = #queries having ref j among their 6 nearest (mask column sums).

Device work per core (512 queries x 4096 refs x 2 clouds):
  s[q,j] = 2*q.x_j - |x_j|^2  (= |q|^2 - dist2; row-constant shift is rank-safe)
  computed as one K=4 matmul with lhsT=[q^T; 1], rhs=[2X^T; -|x|^2].
  top-8 per row via DVE InstMax -> threshold t=6th largest; mask = (s >= t);
  column sums of mask via ones-matmul on PE. Host: Kabsch (3x3 SVD) + O(N) tail.
"""

import os
from contextlib import ExitStack

import numpy as np

import concourse.bass as bass  # noqa: F401  (AP types / plumbing)
import concourse.tile as tile
from concourse import bacc, mybir
from concourse.bass_utils import run_bass_kernel_spmd

N = 4096          # points per cloud
NCORES = 8
NQ = N // NCORES  # 512 queries per core
P = 128           # SBUF partitions
QT = NQ // P      # 4 query tiles per core
CHS = 512         # free-dim chunk = one fp32 PSUM bank
CH = N // CHS     # 8 chunks
L_K = 6

_cache = {}
last_results = None  # test harness reads exec_time_ns off this


def _build_bass():
    nc = bacc.Bacc(
        "TRN2", target_bir_lowering=False, debug=False, num_devices=NCORES
    )
    f32 = mybir.dt.float32
    bf16 = mybir.dt.bfloat16
    # K=11 bf16 hi/lo split of [2*q ; -|x|^2] dot products (see kernel()):
    # rows 0-2 hiQ*hiX2, 3-5 hiQ*loX2, 6-8 loQ*hiX2, 9 one*(-hi_nx), 10 one*(-lo_nx)
    qa_d = nc.dram_tensor("qa", [11, NQ], bf16, kind="ExternalInput")
    rx_d = nc.dram_tensor("rx", [11, N], bf16, kind="ExternalInput")
    ry_d = nc.dram_tensor("ry", [11, N], bf16, kind="ExternalInput")
    cnt_d = nc.dram_tensor("cnt", [1, 2 * N], f32, kind="ExternalOutput")

    with ExitStack() as ctx:
        tc = ctx.enter_context(tile.TileContext(nc))
        const_pool = ctx.enter_context(tc.tile_pool(name="const", bufs=1))
        s_pool = ctx.enter_context(tc.tile_pool(name="s", bufs=4))
        m_pool = ctx.enter_context(tc.tile_pool(name="m", bufs=2 * QT))
        t8_pool = ctx.enter_context(tc.tile_pool(name="t8", bufs=2 * QT))
        ps_pool = ctx.enter_context(tc.tile_pool(name="ps", bufs=4, space="PSUM"))
        cp_pool = ctx.enter_context(tc.tile_pool(name="cp", bufs=2, space="PSUM"))

        qa = const_pool.tile([11, NQ], bf16)
        nc.sync.dma_start(qa[:], qa_d.ap())
        rx = const_pool.tile([11, N], bf16)
        nc.sync.dma_start(rx[:], rx_d.ap())
        ry = const_pool.tile([11, N], bf16)
        nc.sync.dma_start(ry[:], ry_d.ap())
        ones = const_pool.tile([P, 1], bf16)
        nc.vector.memset(ones[:], 1.0)
        out_sb = const_pool.tile([1, 2 * N], f32)

        for ci, r in enumerate((rx, ry)):
            masks = []
            for qt in range(QT):
                s = s_pool.tile([P, N], f32, tag="s")
                mask = m_pool.tile([P, N], bf16, tag="m")
                for ch in range(CH):
                    ps = ps_pool.tile([P, CHS], f32, tag="ps")
                    # bf16 split matmul: full-rate (1 cyc/col) on the PE
                    nc.tensor.matmul(
                        ps[:],
                        qa[:, qt * P : (qt + 1) * P],
                        r[:, ch * CHS : (ch + 1) * CHS],
                        start=True,
                        stop=True,
                    )
                    nc.scalar.copy(s[:, ch * CHS : (ch + 1) * CHS], ps[:])
                t8 = t8_pool.tile([P, 8], f32, tag="t8")
                nc.vector.max(t8[:], s[:])
                # mask = (s >= 6th-largest) as bf16 0/1, on the idle GpSimd
                for ch in range(CH):
                    nc.gpsimd.tensor_scalar(
                        mask[:, ch * CHS : (ch + 1) * CHS],
                        s[:, ch * CHS : (ch + 1) * CHS],
                        t8[:, 5:6],
                        None,
                        mybir.AluOpType.is_ge,
                    )
                masks.append(mask)
            # column sums: cnt[ci, j] = #queries of this core with j in their 6NN
            for ch in range(CH):
                cp = cp_pool.tile([1, CHS], f32, tag="cp")
                for qt in range(QT):
                    nc.tensor.matmul(
                        cp[:],
                        ones[:],
                        masks[qt][:, ch * CHS : (ch + 1) * CHS],
                        start=(qt == 0),
                        stop=(qt == QT - 1),
                    )
                nc.scalar.copy(
                    out_sb[0:1, ci * N + ch * CHS : ci * N + (ch + 1) * CHS], cp[:]
                )
        nc.sync.dma_start(cnt_d.ap(), out_sb[:])

    nc.compile()
    return nc


def _get_nc():
    if "nc" not in _cache:
        _cache["nc"] = _build_bass()
    return _cache["nc"]


def _kabsch_recon(input_t, sf_t):
    """Mirror reference's f32 Kabsch pipeline in numpy; returns rigid_recon [N,3]."""
    pc = np.ascontiguousarray(input_t[0].T.astype(np.float32))  # [N,3]
    recon = pc + np.ascontiguousarray(sf_t[0].T.astype(np.float32))
    cp = pc.mean(axis=0)
    cr = recon.mean(axis=0)
    H = (pc - cp).T @ (recon - cr)
    U, _, Vt = np.linalg.svd(H.astype(np.float64))
    d = np.sign(np.linalg.det(Vt.T @ U.T))
    R = Vt.T @ (np.array([1.0, 1.0, d])[:, None] * U.T)
    t = cr.astype(np.float64) - R @ cp.astype(np.float64)
    return (pc.astype(np.float64) @ R.T + t).astype(np.float32)


def kernel(input_t, sf_t, y1, pred):
    input_t = np.asarray(input_t, dtype=np.float32)
    sf_t = np.asarray(sf_t, dtype=np.float32)
    y1 = np.asarray(y1, dtype=np.float32)
    pred = np.asarray(pred, dtype=np.float32)

    X = _kabsch_recon(input_t, sf_t)                       # rigid_recon [N,3]
    Y = np.ascontiguousarray(y1[0].T.astype(np.float32))   # [N,3]

    import ml_dtypes

    bf = ml_dtypes.bfloat16

    def _split_ref(R):
        # rhs rows for s = 2*q.r - |r|^2 via bf16 hi/lo products
        R2 = (2.0 * R).astype(np.float32)                  # [N,3]
        hiR = R2.astype(bf)
        loR = (R2 - hiR.astype(np.float32)).astype(bf)
        nr = (R.astype(np.float32) ** 2).sum(axis=1, dtype=np.float32)
        hin = nr.astype(bf)
        lon = (nr - hin.astype(np.float32)).astype(bf)
        return np.ascontiguousarray(
            np.concatenate(
                [hiR.T, loR.T, hiR.T, -hin[None, :], -lon[None, :]], axis=0
            ).astype(bf)
        )  # [11, N]

    rx = _split_ref(X)
    ry = _split_ref(Y)

    in_maps = []
    for c in range(NCORES):
        q = X[c * NQ : (c + 1) * NQ].astype(np.float32)    # [NQ,3]
        hiQ = q.astype(bf)
        loQ = (q - hiQ.astype(np.float32)).astype(bf)
        one = np.ones((1, NQ), np.float32).astype(bf)
        qa = np.ascontiguousarray(
            np.concatenate([hiQ.T, hiQ.T, loQ.T, one, one], axis=0).astype(bf)
        )  # [11, NQ]
        in_maps.append({"qa": qa, "rx": rx, "ry": ry})

    nc = _get_nc()
    global last_results
    res = run_bass_kernel_spmd(nc, in_maps, core_ids=list(range(NCORES)))
    last_results = res

    cnt = np.stack([r["cnt"].reshape(2, N) for r in res.results])  # [8, 2, N]
    cx = cnt[:, 0, :].sum(axis=0).astype(np.float64)
    cy = cnt[:, 1, :].sum(axis=0).astype(np.float64)

    Sx = X.astype(np.float64).T @ cx                       # [3]
    Sy = Y.astype(np.float64).T @ cy
    mean_vec = ((Sx - Sy) / ((L_K - 1) * N)).astype(np.float32)

    rigid_refine = X - mean_vec[None, :]
    predT = np.ascontiguousarray(pred[0].T.astype(np.float32))
    loss = np.abs(rigid_refine.astype(np.float64) - predT.astype(np.float64)).mean()
    return np.float32(loss)


# revision 16
# speedup vs baseline: 5.6710x; 5.6710x over previous
"""Trainium2 Bass kernel for nn_ConsistLoss (retrieval_knn).

Math notes
----------
reference() = mean(|rigid_refine - pred^T|) where
  rigid_refine = rigid_recon - mean_i(laplace_x_i - laplace_y_i)
  laplace_c_i  = (sum_{j in 6NN_c(i)} c_j - 6*q_i) / 5       (c in {x=rigid_recon, y})
The -6*q_i terms cancel in (laplace_x - laplace_y), and only the MEAN over all
i is needed, so:
  mean_vec = ( sum_j cx(j)*x_j - sum_j cy(j)*y_j ) / (5*N)
where cx(j) = #queries having ref j among their 6 nearest (mask column sums).

Device work per core (512 queries x 4096 refs x 2 clouds):
  s[q,j] = 2*q.x_j - |x_j|^2  (= |q|^2 - dist2; row-constant shift is rank-safe)
  computed as one K=4 matmul with lhsT=[q^T; 1], rhs=[2X^T; -|x|^2].
  top-8 per row via DVE InstMax -> threshold t=6th largest; mask = (s >= t);
  column sums of mask via ones-matmul on PE. Host: Kabsch (3x3 SVD) + O(N) tail.
"""

import os
from contextlib import ExitStack

import numpy as np

import concourse.bass as bass  # noqa: F401  (AP types / plumbing)
import concourse.tile as tile
from concourse import bacc, mybir
from concourse.bass_utils import run_bass_kernel_spmd

N = 4096          # points per cloud
NCORES = 8
NQ = N // NCORES  # 512 queries per core
P = 128           # SBUF partitions
QT = NQ // P      # 4 query tiles per core
CHS = 512         # free-dim chunk = one fp32 PSUM bank
CH = N // CHS     # 8 chunks
ACT_CH0 = 5       # chunks >= this use the ACT Sign (+-1) mask path
L_K = 6

_cache = {}
last_results = None  # test harness reads exec_time_ns off this


def _build_bass():
    nc = bacc.Bacc(
        "TRN2", target_bir_lowering=False, debug=False, num_devices=NCORES
    )
    f32 = mybir.dt.float32
    bf16 = mybir.dt.bfloat16
    # K=11 bf16 hi/lo split of [2*q ; -|x|^2] dot products (see kernel()):
    # rows 0-2 hiQ*hiX2, 3-5 hiQ*loX2, 6-8 loQ*hiX2, 9 one*(-hi_nx), 10 one*(-lo_nx)
    qa_d = nc.dram_tensor("qa", [11, NQ], bf16, kind="ExternalInput")
    rx_d = nc.dram_tensor("rx", [11, N], bf16, kind="ExternalInput")
    ry_d = nc.dram_tensor("ry", [11, N], bf16, kind="ExternalInput")
    cnt_d = nc.dram_tensor("cnt", [1, 2 * N], f32, kind="ExternalOutput")

    with ExitStack() as ctx:
        tc = ctx.enter_context(tile.TileContext(nc))
        const_pool = ctx.enter_context(tc.tile_pool(name="const", bufs=1))
        s_pool = ctx.enter_context(tc.tile_pool(name="s", bufs=4))
        m_pool = ctx.enter_context(tc.tile_pool(name="m", bufs=2 * QT))
        t8_pool = ctx.enter_context(tc.tile_pool(name="t8", bufs=2 * QT))
        ps_pool = ctx.enter_context(tc.tile_pool(name="ps", bufs=4, space="PSUM"))
        cp_pool = ctx.enter_context(tc.tile_pool(name="cp", bufs=2, space="PSUM"))

        qa = const_pool.tile([11, NQ], bf16)
        nc.sync.dma_start(qa[:], qa_d.ap())
        rx = const_pool.tile([11, N], bf16)
        nc.sync.dma_start(rx[:], rx_d.ap())
        ry = const_pool.tile([11, N], bf16)
        nc.sync.dma_start(ry[:], ry_d.ap())
        ones = const_pool.tile([P, 1], bf16)
        nc.vector.memset(ones[:], 1.0)
        out_sb = const_pool.tile([1, 2 * N], f32)

        for ci, r in enumerate((rx, ry)):
            masks = []
            for qt in range(QT):
                s = s_pool.tile([P, N], f32, tag="s")
                mask = m_pool.tile([P, N], bf16, tag="m")
                for ch in range(CH):
                    ps = ps_pool.tile([P, CHS], f32, tag="ps")
                    # bf16 split matmul: full-rate (1 cyc/col) on the PE
                    nc.tensor.matmul(
                        ps[:],
                        qa[:, qt * P : (qt + 1) * P],
                        r[:, ch * CHS : (ch + 1) * CHS],
                        start=True,
                        stop=True,
                    )
                    nc.scalar.copy(s[:, ch * CHS : (ch + 1) * CHS], ps[:])
                t8 = t8_pool.tile([P, 8], f32, tag="t8")
                nc.vector.max(t8[:], s[:])
                # tp_neg = -(t6+t7)/2: strictly-between threshold for Sign
                tp = t8_pool.tile([P, 1], f32, tag="tp")
                nc.vector.tensor_add(tp[:], t8[:, 5:6], t8[:, 6:7])
                nc.vector.tensor_scalar_mul(tp[:], tp[:], -0.5)
                # mask: top-6 of each row. Chunks 0-4 on DVE as 0/1 via
                # (s >= t6); chunks 5-7 on ACT as -1/+1 via Sign(s - t'),
                # t' strictly between t6 and t7 (host decodes c=(pm+512)/2).
                for ch in range(ACT_CH0):
                    nc.vector.tensor_scalar(
                        mask[:, ch * CHS : (ch + 1) * CHS],
                        s[:, ch * CHS : (ch + 1) * CHS],
                        t8[:, 5:6],
                        None,
                        mybir.AluOpType.is_ge,
                    )
                for ch in range(ACT_CH0, CH):
                    nc.scalar.activation(
                        mask[:, ch * CHS : (ch + 1) * CHS],
                        s[:, ch * CHS : (ch + 1) * CHS],
                        mybir.ActivationFunctionType.Sign,
                        bias=tp[:, 0:1],
                        scale=1.0,
                    )
                masks.append(mask)
            # column sums: cnt[ci, j] = #queries of this core with j in their 6NN
            for ch in range(CH):
                cp = cp_pool.tile([1, CHS], f32, tag="cp")
                for qt in range(QT):
                    nc.tensor.matmul(
                        cp[:],
                        ones[:],
                        masks[qt][:, ch * CHS : (ch + 1) * CHS],
                        start=(qt == 0),
                        stop=(qt == QT - 1),
                    )
                nc.scalar.copy(
                    out_sb[0:1, ci * N + ch * CHS : ci * N + (ch + 1) * CHS], cp[:]
                )
        nc.sync.dma_start(cnt_d.ap(), out_sb[:])

    nc.compile()
    return nc


def _get_nc():
    if "nc" not in _cache:
        _cache["nc"] = _build_bass()
    return _cache["nc"]


def _kabsch_recon(input_t, sf_t):
    """Mirror reference's f32 Kabsch pipeline in numpy; returns rigid_recon [N,3]."""
    pc = np.ascontiguousarray(input_t[0].T.astype(np.float32))  # [N,3]
    recon = pc + np.ascontiguousarray(sf_t[0].T.astype(np.float32))
    cp = pc.mean(axis=0)
    cr = recon.mean(axis=0)
    H = (pc - cp).T @ (recon - cr)
    U, _, Vt = np.linalg.svd(H.astype(np.float64))
    d = np.sign(np.linalg.det(Vt.T @ U.T))
    R = Vt.T @ (np.array([1.0, 1.0, d])[:, None] * U.T)
    t = cr.astype(np.float64) - R @ cp.astype(np.float64)
    return (pc.astype(np.float64) @ R.T + t).astype(np.float32)


def kernel(input_t, sf_t, y1, pred):
    input_t = np.asarray(input_t, dtype=np.float32)
    sf_t = np.asarray(sf_t, dtype=np.float32)
    y1 = np.asarray(y1, dtype=np.float32)
    pred = np.asarray(pred, dtype=np.float32)

    X = _kabsch_recon(input_t, sf_t)                       # rigid_recon [N,3]
    Y = np.ascontiguousarray(y1[0].T.astype(np.float32))   # [N,3]

    import ml_dtypes

    bf = ml_dtypes.bfloat16

    def _split_ref(R):
        # rhs rows for s = 2*q.r - |r|^2 via bf16 hi/lo products
        R2 = (2.0 * R).astype(np.float32)                  # [N,3]
        hiR = R2.astype(bf)
        loR = (R2 - hiR.astype(np.float32)).astype(bf)
        nr = (R.astype(np.float32) ** 2).sum(axis=1, dtype=np.float32)
        hin = nr.astype(bf)
        lon = (nr - hin.astype(np.float32)).astype(bf)
        return np.ascontiguousarray(
            np.concatenate(
                [hiR.T, loR.T, hiR.T, -hin[None, :], -lon[None, :]], axis=0
            ).astype(bf)
        )  # [11, N]

    rx = _split_ref(X)
    ry = _split_ref(Y)

    in_maps = []
    for c in range(NCORES):
        q = X[c * NQ : (c + 1) * NQ].astype(np.float32)    # [NQ,3]
        hiQ = q.astype(bf)
        loQ = (q - hiQ.astype(np.float32)).astype(bf)
        one = np.ones((1, NQ), np.float32).astype(bf)
        qa = np.ascontiguousarray(
            np.concatenate([hiQ.T, hiQ.T, loQ.T, one, one], axis=0).astype(bf)
        )  # [11, NQ]
        in_maps.append({"qa": qa, "rx": rx, "ry": ry})

    nc = _get_nc()
    global last_results
    res = run_bass_kernel_spmd(nc, in_maps, core_ids=list(range(NCORES)))
    last_results = res

    cnt = np.stack([r["cnt"].reshape(2, N) for r in res.results])  # [8, 2, N]
    cnt = cnt.astype(np.float64)
    # chunks >= ACT_CH0 hold +-1 sums over NQ rows: c = (pm + NQ) / 2
    cnt[:, :, ACT_CH0 * CHS :] = (cnt[:, :, ACT_CH0 * CHS :] + NQ) / 2.0
    cx = cnt[:, 0, :].sum(axis=0)
    cy = cnt[:, 1, :].sum(axis=0)

    Sx = X.astype(np.float64).T @ cx                       # [3]
    Sy = Y.astype(np.float64).T @ cy
    mean_vec = ((Sx - Sy) / ((L_K - 1) * N)).astype(np.float32)

    rigid_refine = X - mean_vec[None, :]
    predT = np.ascontiguousarray(pred[0].T.astype(np.float32))
    loss = np.abs(rigid_refine.astype(np.float64) - predT.astype(np.float64)).mean()
    return np.float32(loss)


# revision 26
# speedup vs baseline: 5.6853x; 1.0025x over previous
"""Trainium2 Bass kernel for nn_ConsistLoss (retrieval_knn).

Math notes
----------
reference() = mean(|rigid_refine - pred^T|) where
  rigid_refine = rigid_recon - mean_i(laplace_x_i - laplace_y_i)
  laplace_c_i  = (sum_{j in 6NN_c(i)} c_j - 6*q_i) / 5       (c in {x=rigid_recon, y})
The -6*q_i terms cancel in (laplace_x - laplace_y), and only the MEAN over all
i is needed, so:
  mean_vec = ( sum_j cx(j)*x_j - sum_j cy(j)*y_j ) / (5*N)
where cx(j) = #queries having ref j among their 6 nearest (mask column sums).

Device work per core (512 queries x 4096 refs x 2 clouds):
  s[q,j] = 2*q.x_j - |x_j|^2  (= |q|^2 - dist2; row-constant shift is rank-safe)
  computed as one K=4 matmul with lhsT=[q^T; 1], rhs=[2X^T; -|x|^2].
  top-8 per row via DVE InstMax -> threshold t=6th largest; mask = (s >= t);
  column sums of mask via ones-matmul on PE. Host: Kabsch (3x3 SVD) + O(N) tail.
"""

import os
from contextlib import ExitStack

import numpy as np

import concourse.bass as bass  # noqa: F401  (AP types / plumbing)
import concourse.tile as tile
from concourse import bacc, mybir
from concourse.bass_utils import run_bass_kernel_spmd

N = 4096          # points per cloud
NCORES = 8
NQ = N // NCORES  # 512 queries per core
P = 128           # SBUF partitions
QT = NQ // P      # 4 query tiles per core
CHS = 512         # free-dim chunk = one fp32 PSUM bank
CH = N // CHS     # 8 chunks
ACT_CH0 = 4       # chunks >= this use the ACT Sign (+-1) mask path
DVE_COPY_CH = 2   # chunks < this are PSUM->SBUF copied on DVE, rest on ACT
L_K = 6

_cache = {}
last_results = None  # test harness reads exec_time_ns off this


def _build_bass():
    nc = bacc.Bacc(
        "TRN2", target_bir_lowering=False, debug=False, num_devices=NCORES
    )
    f32 = mybir.dt.float32
    bf16 = mybir.dt.bfloat16
    fp8 = mybir.dt.float8e4
    # K=11 bf16 hi/lo split of [2*q ; -|x|^2] dot products (see kernel()):
    # rows 0-2 hiQ*hiX2, 3-5 hiQ*loX2, 6-8 loQ*hiX2, 9 one*(-hi_nx), 10 one*(-lo_nx)
    qa_d = nc.dram_tensor("qa", [11, NQ], bf16, kind="ExternalInput")
    rx_d = nc.dram_tensor("rx", [11, N], bf16, kind="ExternalInput")
    ry_d = nc.dram_tensor("ry", [11, N], bf16, kind="ExternalInput")
    cnt_d = nc.dram_tensor("cnt", [1, 2 * N], f32, kind="ExternalOutput")

    with ExitStack() as ctx:
        tc = ctx.enter_context(tile.TileContext(nc))
        const_pool = ctx.enter_context(tc.tile_pool(name="const", bufs=1))
        s_pool = ctx.enter_context(tc.tile_pool(name="s", bufs=4))
        m_pool = ctx.enter_context(tc.tile_pool(name="m", bufs=2 * QT))
        t8_pool = ctx.enter_context(tc.tile_pool(name="t8", bufs=2 * QT))
        ps_pool = ctx.enter_context(tc.tile_pool(name="ps", bufs=4, space="PSUM"))
        cp_pool = ctx.enter_context(tc.tile_pool(name="cp", bufs=2, space="PSUM"))

        qa = const_pool.tile([11, NQ], bf16)
        nc.sync.dma_start(qa[:], qa_d.ap())
        rx = const_pool.tile([11, N], bf16)
        nc.sync.dma_start(rx[:], rx_d.ap())
        ry = const_pool.tile([11, N], bf16)
        nc.sync.dma_start(ry[:], ry_d.ap())
        ones = const_pool.tile([P, 1], bf16)
        nc.vector.memset(ones[:], 1.0)
        out_sb = const_pool.tile([1, 2 * N], f32)

        for ci, r in enumerate((rx, ry)):
            masks = []
            for qt in range(QT):
                mask = m_pool.tile([P, N], bf16, tag="m", name=f"m{ci}_{qt}")
                masks.append(mask)
                s = s_pool.tile([P, N], f32, tag="s")
                for ch in range(CH):
                    ps = ps_pool.tile([P, CHS], f32, tag="ps")
                    # bf16 split matmul: full-rate (1 cyc/col) on the PE
                    nc.tensor.matmul(
                        ps[:],
                        qa[:, qt * P : (qt + 1) * P],
                        r[:, ch * CHS : (ch + 1) * CHS],
                        start=True,
                        stop=True,
                    )
                    if ch < DVE_COPY_CH:
                        nc.vector.tensor_copy(s[:, ch * CHS : (ch + 1) * CHS], ps[:])
                    else:
                        nc.scalar.copy(s[:, ch * CHS : (ch + 1) * CHS], ps[:])
                t8 = t8_pool.tile([P, 8], f32, tag="t8")
                nc.vector.max(t8[:], s[:])
                # tp_neg = -(t6+t7)/2: strictly-between threshold for Sign
                tp = t8_pool.tile([P, 1], f32, tag="tp")
                nc.vector.tensor_add(tp[:], t8[:, 5:6], t8[:, 6:7])
                nc.vector.tensor_scalar_mul(tp[:], tp[:], -0.5)
                # mask: top-6 of each row. Low chunks on DVE as 0/1 via
                # (s >= t6); high chunks on ACT as -1/+1 via Sign(s - t'),
                # t' strictly between t6 and t7 (host decodes c=(pm+512)/2).
                for ch in range(ACT_CH0):
                    nc.vector.tensor_scalar(
                        mask[:, ch * CHS : (ch + 1) * CHS],
                        s[:, ch * CHS : (ch + 1) * CHS],
                        t8[:, 5:6],
                        None,
                        mybir.AluOpType.is_ge,
                    )
                for ch in range(ACT_CH0, CH):
                    nc.scalar.activation(
                        mask[:, ch * CHS : (ch + 1) * CHS],
                        s[:, ch * CHS : (ch + 1) * CHS],
                        mybir.ActivationFunctionType.Sign,
                        bias=tp[:, 0:1],
                        scale=1.0,
                    )
            # column sums: cnt[ci, j] = #queries of this core with j in their 6NN
            for ch in range(CH):
                cp = cp_pool.tile([1, CHS], f32, tag="cp")
                for qt in range(QT):
                    nc.tensor.matmul(
                        cp[:],
                        ones[:],
                        masks[qt][:, ch * CHS : (ch + 1) * CHS],
                        start=(qt == 0),
                        stop=(qt == QT - 1),
                    )
                if ch < CH // 2:
                    nc.scalar.copy(
                        out_sb[0:1, ci * N + ch * CHS : ci * N + (ch + 1) * CHS],
                        cp[:],
                    )
                else:
                    nc.vector.tensor_copy(
                        out_sb[0:1, ci * N + ch * CHS : ci * N + (ch + 1) * CHS],
                        cp[:],
                    )
        nc.sync.dma_start(cnt_d.ap(), out_sb[:])

    nc.compile()
    return nc


def _get_nc():
    if "nc" not in _cache:
        _cache["nc"] = _build_bass()
    return _cache["nc"]


def _kabsch_recon(input_t, sf_t):
    """Mirror reference's f32 Kabsch pipeline in numpy; returns rigid_recon [N,3]."""
    pc = np.ascontiguousarray(input_t[0].T.astype(np.float32))  # [N,3]
    recon = pc + np.ascontiguousarray(sf_t[0].T.astype(np.float32))
    cp = pc.mean(axis=0)
    cr = recon.mean(axis=0)
    H = (pc - cp).T @ (recon - cr)
    U, _, Vt = np.linalg.svd(H.astype(np.float64))
    d = np.sign(np.linalg.det(Vt.T @ U.T))
    R = Vt.T @ (np.array([1.0, 1.0, d])[:, None] * U.T)
    t = cr.astype(np.float64) - R @ cp.astype(np.float64)
    return (pc.astype(np.float64) @ R.T + t).astype(np.float32)


def kernel(input_t, sf_t, y1, pred):
    input_t = np.asarray(input_t, dtype=np.float32)
    sf_t = np.asarray(sf_t, dtype=np.float32)
    y1 = np.asarray(y1, dtype=np.float32)
    pred = np.asarray(pred, dtype=np.float32)

    X = _kabsch_recon(input_t, sf_t)                       # rigid_recon [N,3]
    Y = np.ascontiguousarray(y1[0].T.astype(np.float32))   # [N,3]

    import ml_dtypes

    bf = ml_dtypes.bfloat16

    def _split_ref(R):
        # rhs rows for s = 2*q.r - |r|^2 via bf16 hi/lo products
        R2 = (2.0 * R).astype(np.float32)                  # [N,3]
        hiR = R2.astype(bf)
        loR = (R2 - hiR.astype(np.float32)).astype(bf)
        nr = (R.astype(np.float32) ** 2).sum(axis=1, dtype=np.float32)
        hin = nr.astype(bf)
        lon = (nr - hin.astype(np.float32)).astype(bf)
        return np.ascontiguousarray(
            np.concatenate(
                [hiR.T, loR.T, hiR.T, -hin[None, :], -lon[None, :]], axis=0
            ).astype(bf)
        )  # [11, N]

    rx = _split_ref(X)
    ry = _split_ref(Y)

    in_maps = []
    for c in range(NCORES):
        q = X[c * NQ : (c + 1) * NQ].astype(np.float32)    # [NQ,3]
        hiQ = q.astype(bf)
        loQ = (q - hiQ.astype(np.float32)).astype(bf)
        one = np.ones((1, NQ), np.float32).astype(bf)
        qa = np.ascontiguousarray(
            np.concatenate([hiQ.T, hiQ.T, loQ.T, one, one], axis=0).astype(bf)
        )  # [11, NQ]
        in_maps.append({"qa": qa, "rx": rx, "ry": ry})

    nc = _get_nc()
    global last_results
    res = run_bass_kernel_spmd(nc, in_maps, core_ids=list(range(NCORES)))
    last_results = res

    cnt = np.stack([r["cnt"].reshape(2, N) for r in res.results])  # [8, 2, N]
    cnt = cnt.astype(np.float64)
    # chunks >= ACT_CH0 hold +-1 sums over NQ rows: c = (pm + NQ) / 2
    cnt[:, :, ACT_CH0 * CHS :] = (cnt[:, :, ACT_CH0 * CHS :] + NQ) / 2.0
    cx = cnt[:, 0, :].sum(axis=0)
    cy = cnt[:, 1, :].sum(axis=0)

    Sx = X.astype(np.float64).T @ cx                       # [3]
    Sy = Y.astype(np.float64).T @ cy
    mean_vec = ((Sx - Sy) / ((L_K - 1) * N)).astype(np.float32)

    rigid_refine = X - mean_vec[None, :]
    predT = np.ascontiguousarray(pred[0].T.astype(np.float32))
    loss = np.abs(rigid_refine.astype(np.float64) - predT.astype(np.float64)).mean()
    return np.float32(loss)
